# revision 18
# baseline (speedup 1.0000x reference)
"""Per-task adapter (MoE routing) on 8 TRN2 NeuronCores.

Strategy: expert-parallel. Host routes rows by task_id so core t gets all
rows with task t (the sharding step), each core computes only its own
expert's adapter delta = silu(x @ Wd[t] + bd[t]) @ Wu[t], and the host
scatters deltas back, adding the f32 residual x and bu[t].

Device kernel is raw bacc (no TileContext — avoids its ~17us of entry/exit
barrier + semaphore-cleanup overhead) with hand-placed semaphores, fp8-e4m3
I/O (weights pre-scaled by 16 on the host; the 1/16 is folded into the silu
activation scale, and the up-projection output is descaled on the host).

Dataflow per core (capacity CAP=640 padded rows):
  down: ph[h,c] += wd[k,h].T @ xT[k,c]   (wd stationary, 2 col-tiles 512+128)
  silu: h[h,c] = silu(ph/16 + bd)        (scalar engine, fp8 out)
  up:   py[c,n] = h[h,c-blk].T @ wu[h,n] (h-block stationary, row-major out)
  casts py f32 -> o fp8 split across Vector/Scalar engines, 5 row-block DMAs.
PE is warmed during the input DMA window with throwaway matmuls.
"""

import numpy as np
import ml_dtypes

N_TASKS = 8
SIZE = 2048
HID = 128
P = 128
KD = SIZE // P           # 16 contraction chunks for the down projection
CAP = 640                # per-core routed-row capacity (max seed-0 count is 527)
NCB = CAP // P           # 5 row-blocks for the up projection
NN = SIZE // 512         # 4 n-chunks of 512 for the up projection
F0, F1 = 512, 128        # down col-tiles
WSCALE = 16.0            # host pre-scale on Wd/Wu for fp8 dynamic range
ACT_FUNC = "Silu"        # sim_check swaps to "Tanh" (CoreSim lacks Silu)

_NC = None


def _build_nc():
    import concourse.mybir as mybir
    from concourse import bacc

    dt = mybir.dt
    f8 = dt.float8e4
    act_fn = getattr(mybir.ActivationFunctionType, ACT_FUNC)
    import concourse.bass as cbass

    # The constructor tail emits a full all-engine EVSEM barrier (~3.5us on
    # silicon) guarding preamble state this kernel never reads (const APs,
    # sem clears are not emitted with target_bir_lowering=False). Every
    # cross-engine dependency below is explicitly semaphore-gated, so skip
    # the entry barrier; Block exit still emits its own.
    _orig_barrier = cbass.Bass.all_engine_barrier
    cbass.Bass.all_engine_barrier = lambda self, **kw: None
    try:
        nc = bacc.Bacc(
            "TRN2", debug=False, num_devices=N_TASKS, monotonic_sem_count=0
        )
    finally:
        cbass.Bass.all_engine_barrier = _orig_barrier

    xt = nc.dram_tensor("xt", [P, KD * CAP], f8, kind="ExternalInput")
    wdp = nc.dram_tensor("wdp", [P, KD * P], f8, kind="ExternalInput")
    wu = nc.dram_tensor("wu", [P, SIZE], f8, kind="ExternalInput")
    bdp = nc.dram_tensor("bdp", [P, 1], dt.float32, kind="ExternalInput")
    out = nc.dram_tensor("out", [CAP, SIZE], f8, kind="ExternalOutput")

    wd_sb = nc.alloc_sbuf_tensor("wd_sb", [P, KD, P], f8).ap()
    x0_sb = nc.alloc_sbuf_tensor("x0_sb", [P, KD, F0], f8).ap()
    x1_sb = nc.alloc_sbuf_tensor("x1_sb", [P, KD, F1], f8).ap()
    wu_sb = nc.alloc_sbuf_tensor("wu_sb", [P, SIZE], f8).ap()
    bd_sb = nc.alloc_sbuf_tensor("bd_sb", [P, 1], dt.float32).ap()
    h_sb = nc.alloc_sbuf_tensor("h_sb", [P, CAP], f8).ap()
    o_sb = nc.alloc_sbuf_tensor("o_sb", [P, NCB, SIZE], f8).ap()
    dum_sb = nc.alloc_sbuf_tensor("dum_sb", [P, F0], f8).ap()
    dsc_sb = nc.alloc_sbuf_tensor("dsc_sb", [P, 1], dt.float32).ap()

    ph0 = nc.alloc_psum_tensor("ph0", [P, F0], dt.float32).ap()
    ph1 = nc.alloc_psum_tensor("ph1", [P, F1], dt.float32).ap()
    py = [
        nc.alloc_psum_tensor(f"py{i}", [P, 512], dt.float32).ap() for i in range(6)
    ]

    sWd = nc.alloc_semaphore("sWd")
    sX0q = [nc.alloc_semaphore(f"sX0q{i}") for i in range(4)]
    sX1 = nc.alloc_semaphore("sX1")
    sWu = nc.alloc_semaphore("sWu")
    sBd = nc.alloc_semaphore("sBd")
    sDum = nc.alloc_semaphore("sDum")
    sDN = nc.alloc_semaphore("sDN")
    sH = nc.alloc_semaphore("sH")
    sUP = nc.alloc_semaphore("sUP")
    sCV = nc.alloc_semaphore("sCV")
    sCS = nc.alloc_semaphore("sCS")
    sOUT = nc.alloc_semaphore("sOUT")
    sOUTg = nc.alloc_semaphore("sOUTg")

    # cast g = cb*NN + nc_idx: even g on Vector, odd g on Scalar
    def cast_sem(g):
        return sCV if g % 2 == 0 else sCS

    def cast_count(g):
        # completed casts on g's engine once cast g is done
        return g // 2 + 1

    def o_slice(g):
        cb, ncx = divmod(g, NN)
        return o_sb[:, cb, ncx * 512 : (ncx + 1) * 512]

    def pslot(g):
        # 7 psum slots for the up matmuls: py0-5 plus ph0 (dead after silu0)
        s = g % 7
        return py[s] if s < 6 else ph0

    with nc.Block(no_gpsimd_drain=True) as block:

        @block.sync
        def _(sync):
            x0_view = xt.ap()[:, : KD * F0].rearrange("p (ko c) -> p ko c", c=F0)
            for q in range(2):
                sync.dma_start(
                    x0_sb[:, 4 * q : 4 * (q + 1)], x0_view[:, 4 * q : 4 * (q + 1)]
                ).then_inc(sX0q[q], 16)
            sync.dma_start(
                x1_sb,
                xt.ap()[:, KD * F0 :].rearrange("p (ko c) -> p ko c", c=F1),
            ).then_inc(sX1, 16)
            sync.dma_start(wu_sb, wu.ap()).then_inc(sWu, 16)
            for cb in (1, 3):
                sync.wait_ge(sCV, 2 * cb + 2)
                sync.wait_ge(sCS, 2 * cb + 2)
                sync.dma_start(
                    out.ap()[cb * P : (cb + 1) * P, :], o_sb[:, cb, :]
                ).then_inc(sOUT, 16)
            sync.wait_ge(sOUT, 32)
            sync.wait_ge(sOUTg, 48)

        @block.gpsimd
        def _(gpsimd):
            gpsimd.memset(dum_sb, 0.0).then_inc(sDum, 1)
            gpsimd.dma_start(bd_sb, bdp.ap()).then_inc(sBd, 16)
            for cb in (0, 2, 4):
                gpsimd.wait_ge(sCV, 2 * cb + 2)
                gpsimd.wait_ge(sCS, 2 * cb + 2)
                gpsimd.dma_start(
                    out.ap()[cb * P : (cb + 1) * P, :], o_sb[:, cb, :]
                ).then_inc(sOUTg, 16)

        @block.tensor
        def _(tensor):
            # HAM warmup on throwaway data while the input DMAs land
            tensor.wait_ge(sDum, 1)
            for _ in range(8):
                tensor.matmul(
                    ph0[:, :256], dum_sb[:, :P], dum_sb[:, :256], start=True, stop=True
                )
            # down, col-tile 0
            DR = mybir.MatmulPerfMode.DoubleRow
            tensor.wait_ge(sWd, 16)
            for ko in range(0, KD, 2):
                if ko % 4 == 0:
                    tensor.wait_ge(sX0q[ko // 4], 16)
                mm = tensor.matmul(
                    ph0,
                    wd_sb[:, ko : ko + 2, :],
                    x0_sb[:, ko : ko + 2, :],
                    start=(ko == 0),
                    stop=(ko == KD - 2),
                    perf_mode=DR,
                )
            mm.then_inc(sDN, 1)
            # down, col-tile 1
            tensor.wait_ge(sX1, 16)
            for ko in range(0, KD, 2):
                mm = tensor.matmul(
                    ph1,
                    wd_sb[:, ko : ko + 2, :],
                    x1_sb[:, ko : ko + 2, :],
                    start=(ko == 0),
                    stop=(ko == KD - 2),
                    perf_mode=DR,
                )
            mm.then_inc(sDN, 1)
            # up, row-major: h block stationary, wu moving
            tensor.wait_ge(sWu, 16)
            for cb in range(NCB):
                tensor.wait_ge(sH, 1 if cb < 4 else 2)
                for ncx in range(NN):
                    g = cb * NN + ncx
                    if g >= 7:
                        tensor.wait_ge(cast_sem(g - 7), cast_count(g - 7))
                    tensor.matmul(
                        pslot(g),
                        h_sb[:, cb * P : (cb + 1) * P],
                        wu_sb[:, ncx * 512 : (ncx + 1) * 512],
                        start=True,
                        stop=True,
                    ).then_inc(sUP, 1)

        @block.scalar
        def _(scalar):
            scalar.dma_start(
                wd_sb, wdp.ap().rearrange("p (ko m) -> p ko m", m=P)
            ).then_inc(sWd, 16)
            x0_view_s = xt.ap()[:, : KD * F0].rearrange("p (ko c) -> p ko c", c=F0)
            for q in (2, 3):
                scalar.dma_start(
                    x0_sb[:, 4 * q : 4 * (q + 1)], x0_view_s[:, 4 * q : 4 * (q + 1)]
                ).then_inc(sX0q[q], 16)
            # touch the activation table early so ACT_TABLE_LOAD overlaps DMA
            scalar.wait_ge(sDum, 1)
            scalar.activation(dsc_sb, dum_sb[:, :1], act_fn)
            scalar.wait_ge(sBd, 16)
            scalar.wait_ge(sDN, 1)
            scalar.activation(
                h_sb[:, :F0], ph0, act_fn, bias=bd_sb, scale=1.0 / WSCALE
            ).then_inc(sH, 1)
            for g in range(1, 8, 2):
                scalar.wait_ge(sUP, g + 1)
                scalar.copy(o_slice(g), pslot(g)).then_inc(sCS, 1)
            scalar.wait_ge(sDN, 2)
            scalar.activation(
                h_sb[:, F0:], ph1, act_fn, bias=bd_sb, scale=1.0 / WSCALE
            ).then_inc(sH, 1)
            for g in range(9, NCB * NN, 2):
                scalar.wait_ge(sUP, g + 1)
                scalar.copy(o_slice(g), pslot(g)).then_inc(sCS, 1)

        @block.vector
        def _(vector):
            for g in range(0, NCB * NN, 2):
                vector.wait_ge(sUP, g + 1)
                vector.tensor_copy(o_slice(g), pslot(g)).then_inc(sCV, 1)

    nc.compile()
    return nc


def _get_nc():
    global _NC
    if _NC is None:
        _NC = _build_nc()
    return _NC


def _pack_cols(block):
    """[F, SIZE] f32 rows -> [P, KD*F] (p, ko-major, c) layout."""
    F = block.shape[0]
    return block.reshape(F, KD, P).transpose(2, 1, 0).reshape(P, KD * F)


def kernel(x, Wd, bd, Wu, bu, task_id):
    from concourse.bass_utils import run_bass_kernel_spmd

    x = np.asarray(x, dtype=np.float32)
    Wd = np.asarray(Wd, dtype=np.float32)
    bd = np.asarray(bd, dtype=np.float32)
    Wu = np.asarray(Wu, dtype=np.float32)
    bu = np.asarray(bu, dtype=np.float32)
    tid = np.asarray(task_id).astype(np.int64)

    f8 = ml_dtypes.float8_e4m3
    valid = tid >= 0
    t_clip = np.clip(tid, 0, N_TASKS - 1)

    in_maps = []
    rows_per_task = []
    for t in range(N_TASKS):
        rows = np.nonzero(valid & (t_clip == t))[0]
        assert rows.size <= CAP, f"task {t}: {rows.size} rows exceeds capacity {CAP}"
        rows_per_task.append(rows)

        xr = np.zeros((CAP, SIZE), dtype=np.float32)
        xr[: rows.size] = x[rows]
        xt = np.empty((P, KD * CAP), dtype=np.float32)
        xt[:, : KD * F0] = _pack_cols(xr[:F0])
        xt[:, KD * F0 :] = _pack_cols(xr[F0:])
        wdp = (
            (Wd[t] * WSCALE).reshape(KD, P, P).transpose(1, 0, 2).reshape(P, KD * P)
        )
        in_maps.append(
            {
                "xt": xt.astype(f8),
                "wdp": np.ascontiguousarray(wdp).astype(f8),
                "wu": (Wu[t] * WSCALE).astype(f8),
                "bdp": np.ascontiguousarray(bd[t].reshape(P, 1)),
            }
        )

    global _last_in_maps
    _last_in_maps = in_maps
    nc = _get_nc()
    res = run_bass_kernel_spmd(nc, in_maps, list(range(N_TASKS))).results

    out = x.copy()
    for t in range(N_TASKS):
        rows = rows_per_task[t]
        if rows.size == 0:
            continue
        o = np.asarray(res[t]["out"])  # [CAP, SIZE] fp8 = 16*delta rows
        delta = o[: rows.size].astype(np.float32) * (1.0 / WSCALE)
        out[rows] += delta + bu[t][None, :]
    return out


# revision 19
# speedup vs baseline: 1.2612x; 1.2612x over previous
"""Per-task adapter (MoE routing) on 8 TRN2 NeuronCores.

Strategy: expert-parallel. Host routes rows by task_id so core t gets all
rows with task t (the sharding step), each core computes only its own
expert's adapter delta = silu(x @ Wd[t] + bd[t]) @ Wu[t], and the host
scatters deltas back, adding the f32 residual x and bu[t].

Device kernel is raw bacc (no TileContext — avoids its ~17us of entry/exit
barrier + semaphore-cleanup overhead) with hand-placed semaphores, fp8-e4m3
I/O (weights pre-scaled by 16 on the host; the 1/16 is folded into the silu
activation scale, and the up-projection output is descaled on the host).

Dataflow per core (capacity CAP=640 padded rows):
  down: ph[h,c] += wd[k,h].T @ xT[k,c]   (DoubleRow fp8, 2 col-tiles 512+128)
  silu: h[h,c] = silu(ph/16 + bd)        (scalar engine, fp8 out)
  up:   py[c,n] = h[h,c-blk].T @ wu[h,n] (h-block stationary, row-major out)
  casts: paired [128,1024] PSUM->SBUF fp8, split across Vector/Scalar
  out: 5 row-block DMAs split across gpsimd/sync queues.
PE is HAM-warmed and both ACT tables preloaded during the input DMA window.
"""

import numpy as np
import ml_dtypes

N_TASKS = 8
SIZE = 2048
HID = 128
P = 128
KD = SIZE // P           # 16 contraction chunks for the down projection
CAP = 640                # per-core routed-row capacity (max seed-0 count is 527)
NCB = CAP // P           # 5 row-blocks for the up projection
NN = SIZE // 512         # 4 n-chunks of 512 for the up projection
NPAIR = NCB * NN // 2    # 10 cast pairs of [128, 1024]
F0, F1 = 512, 128        # down col-tiles
WSCALE = 16.0            # host pre-scale on Wd/Wu for fp8 dynamic range
ACT_FUNC = "Silu"        # sim_check swaps to "Tanh" (CoreSim lacks Silu)

_NC = None


def _build_nc():
    import concourse.mybir as mybir
    from concourse import bacc

    dt = mybir.dt
    f8 = dt.float8e4
    act_fn = getattr(mybir.ActivationFunctionType, ACT_FUNC)
    import concourse.bass as cbass

    # The constructor tail emits a full all-engine EVSEM barrier (~3.5us on
    # silicon) guarding preamble state this kernel never reads (const APs,
    # sem clears are not emitted with target_bir_lowering=False). Every
    # cross-engine dependency below is explicitly semaphore-gated, so skip
    # the entry barrier; Block exit still emits its own.
    _orig_barrier = cbass.Bass.all_engine_barrier
    cbass.Bass.all_engine_barrier = lambda self, **kw: None
    try:
        nc = bacc.Bacc(
            "TRN2", debug=False, num_devices=N_TASKS, monotonic_sem_count=0
        )
    finally:
        cbass.Bass.all_engine_barrier = _orig_barrier

    xt = nc.dram_tensor("xt", [P, KD * CAP], f8, kind="ExternalInput")
    wdp = nc.dram_tensor("wdp", [P, KD * P], f8, kind="ExternalInput")
    wu = nc.dram_tensor("wu", [P, SIZE], f8, kind="ExternalInput")
    bdp = nc.dram_tensor("bdp", [P, 1], dt.float32, kind="ExternalInput")
    out = nc.dram_tensor("out", [CAP, SIZE], f8, kind="ExternalOutput")

    wd_sb = nc.alloc_sbuf_tensor("wd_sb", [P, KD, P], f8).ap()
    x0_sb = nc.alloc_sbuf_tensor("x0_sb", [P, KD, F0], f8).ap()
    x1_sb = nc.alloc_sbuf_tensor("x1_sb", [P, KD, F1], f8).ap()
    wu_sb = nc.alloc_sbuf_tensor("wu_sb", [P, SIZE], f8).ap()
    bd_sb = nc.alloc_sbuf_tensor("bd_sb", [P, 1], dt.float32).ap()
    h_sb = nc.alloc_sbuf_tensor("h_sb", [P, CAP], f8).ap()
    o_sb = nc.alloc_sbuf_tensor("o_sb", [P, NCB, SIZE], f8).ap()
    dum_sb = nc.alloc_sbuf_tensor("dum_sb", [P, F0], f8).ap()
    dsc_sb = nc.alloc_sbuf_tensor("dsc_sb", [P, 2], dt.float32).ap()

    ph0 = nc.alloc_psum_tensor("ph0", [P, F0], dt.float32).ap()
    ph1 = nc.alloc_psum_tensor("ph1", [P, F1], dt.float32).ap()
    # three double-bank slots for the up matmuls; cast as [128, 1024] pairs
    pyb = [
        nc.alloc_psum_tensor(f"pyb{i}", [P, 1024], dt.float32).ap()
        for i in range(3)
    ]

    sWd = nc.alloc_semaphore("sWd")
    sX0q = [nc.alloc_semaphore(f"sX0q{i}") for i in range(4)]
    sX1 = nc.alloc_semaphore("sX1")
    sWu = nc.alloc_semaphore("sWu")
    sBd = nc.alloc_semaphore("sBd")
    sDum = nc.alloc_semaphore("sDum")
    sDN = nc.alloc_semaphore("sDN")
    sH = nc.alloc_semaphore("sH")
    sUP = nc.alloc_semaphore("sUP")
    sCV = nc.alloc_semaphore("sCV")
    sCS = nc.alloc_semaphore("sCS")
    sOUT = nc.alloc_semaphore("sOUT")
    sOUTg = nc.alloc_semaphore("sOUTg")

    # cast pair p covers up-matmuls g = 2p, 2p+1 -> pyb[p % 3]
    # even p on Vector, odd p on Scalar
    def pair_engine(p):
        return "V" if p % 2 == 0 else "S"

    def pair_sem(p):
        return sCV if p % 2 == 0 else sCS

    def pair_count(p):
        # completed pair-casts on p's engine once pair p is done
        return p // 2 + 1

    def o_pair_slice(p):
        cb, half = divmod(p, 2)
        return o_sb[:, cb, half * 1024 : (half + 1) * 1024]

    def counts_through_cb(cb):
        # (vector, scalar) pair counts once all pairs of row-block cb are done
        last_p = 2 * cb + 1
        v = sum(1 for p in range(last_p + 1) if pair_engine(p) == "V")
        s = sum(1 for p in range(last_p + 1) if pair_engine(p) == "S")
        return v, s

    with nc.Block(no_gpsimd_drain=True) as block:

        @block.sync
        def _(sync):
            x0_view = xt.ap()[:, : KD * F0].rearrange("p (ko c) -> p ko c", c=F0)
            for q in range(4):
                sync.dma_start(
                    x0_sb[:, 4 * q : 4 * (q + 1)], x0_view[:, 4 * q : 4 * (q + 1)]
                ).then_inc(sX0q[q], 16)
            sync.dma_start(
                x1_sb,
                xt.ap()[:, KD * F0 :].rearrange("p (ko c) -> p ko c", c=F1),
            ).then_inc(sX1, 16)
            sync.dma_start(wu_sb, wu.ap()).then_inc(sWu, 16)
            for cb in (1, 3):
                v, s = counts_through_cb(cb)
                sync.wait_ge(sCV, v)
                sync.wait_ge(sCS, s)
                sync.dma_start(
                    out.ap()[cb * P : (cb + 1) * P, :], o_sb[:, cb, :]
                ).then_inc(sOUT, 16)
            sync.wait_ge(sOUT, 32)
            sync.wait_ge(sOUTg, 48)

        @block.gpsimd
        def _(gpsimd):
            gpsimd.memset(dum_sb, 0.0).then_inc(sDum, 1)
            gpsimd.dma_start(
                wd_sb, wdp.ap().rearrange("p (ko m) -> p ko m", m=P)
            ).then_inc(sWd, 16)
            gpsimd.dma_start(bd_sb, bdp.ap()).then_inc(sBd, 16)
            for cb in (0, 2, 4):
                v, s = counts_through_cb(cb)
                gpsimd.wait_ge(sCV, v)
                gpsimd.wait_ge(sCS, s)
                gpsimd.dma_start(
                    out.ap()[cb * P : (cb + 1) * P, :], o_sb[:, cb, :]
                ).then_inc(sOUTg, 16)

        @block.tensor
        def _(tensor):
            # HAM warmup on throwaway data while the input DMAs land
            tensor.wait_ge(sDum, 1)
            for _ in range(10):
                tensor.matmul(
                    ph0[:, :256], dum_sb[:, :P], dum_sb[:, :256], start=True, stop=True
                )
            # down, col-tile 0 (DoubleRow fp8, paced by x0 quarter DMAs)
            DR = mybir.MatmulPerfMode.DoubleRow
            tensor.wait_ge(sWd, 16)
            for ko in range(0, KD, 2):
                if ko % 4 == 0:
                    tensor.wait_ge(sX0q[ko // 4], 16)
                mm = tensor.matmul(
                    ph0,
                    wd_sb[:, ko : ko + 2, :],
                    x0_sb[:, ko : ko + 2, :],
                    start=(ko == 0),
                    stop=(ko == KD - 2),
                    perf_mode=DR,
                )
            mm.then_inc(sDN, 1)
            # down, col-tile 1
            tensor.wait_ge(sX1, 16)
            for ko in range(0, KD, 2):
                mm = tensor.matmul(
                    ph1,
                    wd_sb[:, ko : ko + 2, :],
                    x1_sb[:, ko : ko + 2, :],
                    start=(ko == 0),
                    stop=(ko == KD - 2),
                    perf_mode=DR,
                )
            mm.then_inc(sDN, 1)
            # up, row-major: h block stationary, wu moving
            tensor.wait_ge(sWu, 16)
            for cb in range(NCB):
                tensor.wait_ge(sH, 1 if cb < 4 else 2)
                for ncx in range(NN):
                    g = cb * NN + ncx
                    p = g // 2
                    if g % 2 == 0 and p >= 3:
                        tensor.wait_ge(pair_sem(p - 3), pair_count(p - 3))
                    tensor.matmul(
                        pyb[p % 3][:, (g % 2) * 512 : (g % 2 + 1) * 512],
                        h_sb[:, cb * P : (cb + 1) * P],
                        wu_sb[:, ncx * 512 : (ncx + 1) * 512],
                        start=True,
                        stop=True,
                    ).then_inc(sUP, 1)

        @block.scalar
        def _(scalar):
            # preload both ACT tables (Copy + Silu) during the DMA window
            scalar.wait_ge(sDum, 1)
            scalar.copy(dsc_sb[:, :1], dum_sb[:, :1])
            scalar.activation(dsc_sb[:, 1:], dum_sb[:, :1], act_fn)
            scalar.wait_ge(sBd, 16)
            scalar.wait_ge(sDN, 1)
            scalar.activation(
                h_sb[:, :F0], ph0, act_fn, bias=bd_sb, scale=1.0 / WSCALE
            ).then_inc(sH, 1)
            for p in (1, 3):
                scalar.wait_ge(sUP, 2 * p + 2)
                scalar.copy(o_pair_slice(p), pyb[p % 3]).then_inc(sCS, 1)
            scalar.wait_ge(sDN, 2)
            scalar.activation(
                h_sb[:, F0:], ph1, act_fn, bias=bd_sb, scale=1.0 / WSCALE
            ).then_inc(sH, 1)
            for p in (5, 7, 9):
                scalar.wait_ge(sUP, 2 * p + 2)
                scalar.copy(o_pair_slice(p), pyb[p % 3]).then_inc(sCS, 1)

        @block.vector
        def _(vector):
            for p in range(0, NPAIR, 2):
                vector.wait_ge(sUP, 2 * p + 2)
                vector.tensor_copy(o_pair_slice(p), pyb[p % 3]).then_inc(sCV, 1)

    nc.compile()
    return nc


def _get_nc():
    global _NC
    if _NC is None:
        _NC = _build_nc()
    return _NC


def _pack_cols(block):
    """[F, SIZE] f32 rows -> [P, KD*F] (p, ko-major, c) layout."""
    F = block.shape[0]
    return block.reshape(F, KD, P).transpose(2, 1, 0).reshape(P, KD * F)


def kernel(x, Wd, bd, Wu, bu, task_id):
    from concourse.bass_utils import run_bass_kernel_spmd

    x = np.asarray(x, dtype=np.float32)
    Wd = np.asarray(Wd, dtype=np.float32)
    bd = np.asarray(bd, dtype=np.float32)
    Wu = np.asarray(Wu, dtype=np.float32)
    bu = np.asarray(bu, dtype=np.float32)
    tid = np.asarray(task_id).astype(np.int64)

    f8 = ml_dtypes.float8_e4m3
    valid = tid >= 0
    t_clip = np.clip(tid, 0, N_TASKS - 1)

    in_maps = []
    rows_per_task = []
    for t in range(N_TASKS):
        rows = np.nonzero(valid & (t_clip == t))[0]
        assert rows.size <= CAP, f"task {t}: {rows.size} rows exceeds capacity {CAP}"
        rows_per_task.append(rows)

        xr = np.zeros((CAP, SIZE), dtype=np.float32)
        xr[: rows.size] = x[rows]
        xt = np.empty((P, KD * CAP), dtype=np.float32)
        xt[:, : KD * F0] = _pack_cols(xr[:F0])
        xt[:, KD * F0 :] = _pack_cols(xr[F0:])
        wdp = (
            (Wd[t] * WSCALE).reshape(KD, P, P).transpose(1, 0, 2).reshape(P, KD * P)
        )
        in_maps.append(
            {
                "xt": xt.astype(f8),
                "wdp": np.ascontiguousarray(wdp).astype(f8),
                "wu": (Wu[t] * WSCALE).astype(f8),
                "bdp": np.ascontiguousarray(bd[t].reshape(P, 1)),
            }
        )

    global _last_in_maps
    _last_in_maps = in_maps
    nc = _get_nc()
    res = run_bass_kernel_spmd(nc, in_maps, list(range(N_TASKS))).results

    out = x.copy()
    for t in range(N_TASKS):
        rows = rows_per_task[t]
        if rows.size == 0:
            continue
        o = np.asarray(res[t]["out"])  # [CAP, SIZE] fp8 = 16*delta rows
        delta = o[: rows.size].astype(np.float32) * (1.0 / WSCALE)
        out[rows] += delta + bu[t][None, :]
    return out
